# revision 29
# baseline (speedup 1.0000x reference)
"""Multi-head attention (B=2, S=2048, H=1024, 16 heads) on 8 NeuronCores.

Hybrid tensor-parallel sharding: core c handles batch c//4 and head-group
c%4 (4 heads).  Each core computes QKV for its heads over its batch, full
attention, and a partial output projection (its 256 rows of w_dense).  The
host sums the 4 partials per batch (the all-reduce) and adds the
output-side bias terms.

All matmul operands are fp16 (host pre-converts; ~8x the mantissa of bf16
at the same PE speed).  Accumulation is fp32 in PSUM.

Per-core layout:
  hsT  [128, 8, 512]    hidden states transposed (PE transpose), per
                        512-seq window.
  QT   [128, 2, 2048]   q transposed, packed: pair p rows = [h_even | h_odd].
  KTz  [128, 4, 2048]   k transposed per head, zero-padded so each head
                        contracts against the packed QT pair.
  Vn   [128, 16, 4, 66] v natural: partition = seq within 128-chunk;
                        col 64 is 1.0 so P@V also emits softmax rowsums.
  PT   [128, 4, 1024]   exp(scores) ring: partition = k within chunk.
                        3 of 4 tiles via ScalarE exp; every 4th via a
                        DVE Schraudolph fast-exp (int16 bit trick).
  ctxT [128, 2, 2048]   normalized context, packed like QT: the dense
                        matmul contracts full 128-row pair chunks.
  Normalization: P@V rowsum row -> reciprocal -> gpsimd partition
  broadcast -> one DVE multiply per psum tile (fuses the PSUM->SBUF copy).
A burst of dummy matmuls at t=0 warms the PE HAM clock gate while the
first DMAs land.
"""

import os
import sys
import types

sys.path.insert(0, "/opt/trn_rl_repo")

import numpy as np


def _install_ntff_shim():
    """The trimmed container image lacks ``antenv.axon_hooks``, which
    ``run_bass_kernel_spmd(trace=True)`` needs to capture NTFF profiles
    under axon.  Recreate it from the boot helper + the injected .so."""
    if "antenv.axon_hooks" in sys.modules:
        return
    try:
        from trn_agent_boot.trn_boot import _ntff_profile_via_ctypes
        so = "/opt/axon/libaxon_pjrt.so"
        if not os.path.exists(so):
            return
        hook = _ntff_profile_via_ctypes(so)
        mod = types.ModuleType("antenv.axon_hooks")
        mod.get_axon_ntff_profile_hook = lambda: hook
        mod.set_axon_ntff_profile_hook = lambda h: None
        sys.modules["antenv.axon_hooks"] = mod
    except Exception:
        pass


_install_ntff_shim()

import concourse.bass as bass
import concourse.mybir as mybir
import concourse.tile as tile
from concourse import bacc
from concourse.bass_utils import run_bass_kernel_spmd
from concourse.masks import make_identity

F32 = mybir.dt.float32
F16 = mybir.dt.float16
I16 = mybir.dt.int16
EXP = mybir.ActivationFunctionType.Exp
MULT = mybir.AluOpType.mult
ADD = mybir.AluOpType.add

B, S, HID = 2, 2048, 1024
HEADS, D = 16, 64
NCORES = 8
NBG = 2                          # batch groups
NHG = NCORES // NBG              # head groups = 4
HPC = HEADS // NHG               # heads per core = 4
NPAIR = HPC // 2                 # head pairs per core = 2
CW = HPC * D                     # per-core ctx width = 256
NHB = HID // 128                 # hidden 128-chunks = 8
WSEQ = 512                       # seq window for transpose+QKV
NWIN = S // WSEQ                 # 4
QW = 1024                        # q window in attention
NKT = S // 128                   # k chunks = 16
RING = 4

# fp16 Schraudolph fast-exp (DVE): i16 = round(x*SCH_A + SCH_B), bits
# reinterpreted as fp16 give exp(0.125*x) to ~3% max rel err.
SCH_A = 1024.0 * 1.4426950408889634 * 0.125   # 184.66496523378733
SCH_B = 15360.0 - 44.74                        # calibrated bias


def build_nc():
    nc = bacc.Bacc("TRN2", target_bir_lowering=False, debug=False,
                   num_devices=NCORES)

    hst = nc.dram_tensor("hst", [HID, S], F16, kind="ExternalInput")
    wq = nc.dram_tensor("wq", [HID, CW], F16, kind="ExternalInput")
    wk = nc.dram_tensor("wk", [HID, CW], F16, kind="ExternalInput")
    wv = nc.dram_tensor("wv", [HID, CW], F16, kind="ExternalInput")
    bq = nc.dram_tensor("bq", [CW, 1], F32, kind="ExternalInput")
    bk = nc.dram_tensor("bk", [CW, 1], F32, kind="ExternalInput")
    wd = nc.dram_tensor("wd", [CW, HID], F16, kind="ExternalInput")
    out = nc.dram_tensor("out", [S, HID], F32, kind="ExternalOutput")

    with tile.TileContext(nc) as tc:
        with (
            tc.tile_pool(name="persist", bufs=1) as pp,
            tc.tile_pool(name="pt", bufs=1) as ptp,
        ):
            # warmup scratch memset is the first gpsimd instruction so the
            # HAM-warming dummy matmuls can issue right after the preamble
            wu = pp.tile([128, 640], F16)
            nc.gpsimd.memset(wu[:], 0.0)

            ident = pp.tile([128, 128], F32)
            make_identity(nc, ident[:])
            identh_t = pp.tile([128, 128], F16)
            nc.vector.tensor_copy(identh_t[:], ident[:])
            identh = identh_t[:]

            wq_sb = pp.tile([128, NHB, CW], F16)
            wk_sb = pp.tile([128, NHB, CW], F16)
            wv_sb = pp.tile([128, NHB, CW], F16)
            for wsb, wdr in ((wq_sb, wq), (wk_sb, wk), (wv_sb, wv)):
                nc.gpsimd.dma_start(
                    wsb[:], wdr.ap().rearrange("(c p) m -> p c m", p=128))
            bq_sb = pp.tile([128, NPAIR], F32)
            bk_sb = pp.tile([128, NPAIR], F32)
            nc.gpsimd.dma_start(
                bq_sb[:], bq.ap().rearrange("(r p) m -> p (r m)", p=128))
            nc.gpsimd.dma_start(
                bk_sb[:], bk.ap().rearrange("(r p) m -> p (r m)", p=128))
            wd_sb = pp.tile([128, NPAIR, HID], F16)
            nc.gpsimd.dma_start(
                wd_sb[:], wd.ap().rearrange("(r p) m -> p r m", p=128))

            QT = pp.tile([128, NPAIR, S], F16)
            KTz = pp.tile([128, HPC, S], F16)
            Vn = pp.tile([128, NKT, HPC, 66], F16)
            ctxT = pp.tile([128, NPAIR, S], F16)
            PT = ptp.tile([128, RING, QW], F16)

            # zero-pad KTz: even head -> rows 64-127 zero, odd head ->
            # rows 0-63 zero; ones column for the P@V rowsum trick
            for h in range(HPC):
                if h % 2 == 0:
                    nc.vector.memset(KTz[D:128, h, :], 0.0)
                else:
                    nc.vector.memset(KTz[0:D, h, :], 0.0)
            nc.vector.memset(Vn[:, :, :, 64:65], 1.0)

            # ---------------- phase 1: load hs^T + QKV projections -------
            hsTf = pp.tile([128, NHB, S], F16)
            hst_re = hst.ap().rearrange("(c p) s -> p c s", p=128)
            with (
                tc.tile_pool(name="warm", bufs=1,
                             space=bass.MemorySpace.PSUM) as wup,
                tc.tile_pool(name="warms", bufs=1) as wsp,
                tc.tile_pool(name="vtw", bufs=2) as vwp,
                tc.tile_pool(name="ps_tr", bufs=2,
                             space=bass.MemorySpace.PSUM) as ptr,
                tc.tile_pool(name="ps_qkv", bufs=2,
                             space=bass.MemorySpace.PSUM) as pqk,
            ):
                # dummy matmuls at t=0: warm the HAM clock gate while the
                # first hs window DMAs in (PE is otherwise idle and cold)
                wups = wup.tile([128, 512], F32)
                for _ in range(40):
                    nc.tensor.matmul(wups[:], wu[:, 0:128], wu[:, 128:640],
                                     start=True, stop=True)

                for w in range(NWIN):
                    r0 = w * WSEQ
                    wsl = slice(r0, r0 + WSEQ)
                    nc.sync.dma_start(hsTf[:, :, wsl], hst_re[:, :, wsl])
                    for tgt in range(3):
                        for pr in range(NPAIR):
                            csl = slice(pr * 128, (pr + 1) * 128)
                            ps = pqk.tile([128, WSEQ], F32, tag="qkv")
                            wsb = (wq_sb, wk_sb, wv_sb)[tgt]
                            for hb in range(NHB):
                                nc.tensor.matmul(
                                    ps[:], wsb[:, hb, csl],
                                    hsTf[:, hb, wsl],
                                    start=(hb == 0), stop=(hb == NHB - 1))
                            if tgt == 0:
                                nc.vector.tensor_scalar_add(
                                    QT[:, pr, wsl], ps[:],
                                    bq_sb[:, pr:pr + 1])
                            elif tgt == 1:
                                nc.vector.tensor_scalar_add(
                                    KTz[0:D, 2 * pr, wsl], ps[0:D, :],
                                    bk_sb[0:D, pr:pr + 1])
                                nc.vector.tensor_scalar_add(
                                    KTz[D:128, 2 * pr + 1, wsl],
                                    ps[D:128, :],
                                    bk_sb[D:128, pr:pr + 1])
                            else:
                                vtw = vwp.tile([128, WSEQ], F16)
                                nc.vector.tensor_copy(vtw[:], ps[:])
                                vps = ptr.tile([128, WSEQ], F16, tag="vtr")
                                for sb2 in range(WSEQ // 128):
                                    nc.tensor.transpose(
                                        vps[:, sb2 * 128:(sb2 + 1) * 128],
                                        vtw[:, sb2 * 128:(sb2 + 1) * 128],
                                        identh)
                                ch0 = r0 // 128
                                nc.vector.tensor_copy(
                                    Vn[:, ch0:ch0 + 4,
                                       2 * pr:2 * pr + 2, 0:64],
                                    vps[:].rearrange(
                                        "p (c h d) -> p c h d", c=4, h=2))

            # ---------------- phase 2: attention + output projection -----
            with (
                tc.tile_pool(name="ps_st", bufs=1,
                             space=bass.MemorySpace.PSUM) as pst,
                tc.tile_pool(name="ps_pv", bufs=1,
                             space=bass.MemorySpace.PSUM) as ppv,
                tc.tile_pool(name="dens", bufs=2) as dnp,
                tc.tile_pool(name="outst", bufs=4) as osp,
            ):
                def dense_block(dqw):
                    # packed pair contraction, out natural [q, oc]
                    dqbase = dqw * QW
                    for qc in range(QW // 128):
                        ssl = slice(dqbase + qc * 128,
                                    dqbase + (qc + 1) * 128)
                        for nt in range(2):
                            nsl = slice(nt * 512, (nt + 1) * 512)
                            dpw = pst.tile([128, QW], F32, tag="st", bufs=2)
                            dps = dpw[:, 0:512]
                            for pr in range(NPAIR):
                                nc.tensor.matmul(
                                    dps, ctxT[:, pr, ssl],
                                    wd_sb[:, pr, nsl],
                                    start=(pr == 0), stop=(pr == NPAIR - 1))
                            ob = osp.tile([128, 512], F32)
                            nc.vector.tensor_copy(ob[:], dps)
                            nc.sync.dma_start(out[ssl, nsl], ob[:])

                def normalize(qbase, h, pva, pvb):
                    # softmax denominators: rowsum rows -> approx
                    # reciprocal -> gpsimd partition broadcast -> one DVE
                    # multiply per psum tile (fuses the PSUM->SBUF copy).
                    # Emitted AFTER the next head's attention so the DVE
                    # FIFO never stalls the exp pipeline on this chain.
                    pr = h // 2
                    hr = slice((h % 2) * D, (h % 2) * D + D)
                    den = dnp.tile([1, QW], F32, tag="den")
                    nc.vector.tensor_copy(den[0:1, 0:512], pva[D:D + 1, :])
                    nc.vector.tensor_copy(den[0:1, 512:QW], pvb[D:D + 1, :])
                    rden = dnp.tile([1, QW], F32, tag="rden")
                    nc.vector.reciprocal_approx_fast(rden[:], den[:])
                    rbc = dnp.tile([D, QW], F32, tag="rbc")
                    nc.gpsimd.partition_broadcast(
                        rbc[:], rden[0:1, :], channels=D)
                    for qh, pvh in ((0, pva), (1, pvb)):
                        nc.vector.tensor_tensor(
                            ctxT[hr, pr, qbase + qh * 512:
                                 qbase + (qh + 1) * 512],
                            pvh[0:D, :],
                            rbc[:, qh * 512:(qh + 1) * 512],
                            op=MULT)

                pending = None
                for qw in range(S // QW):
                    qbase = qw * QW
                    for h in range(HPC):
                        pr = h // 2
                        hr = slice((h % 2) * D, (h % 2) * D + D)
                        pva = ppv.tile([D + 1, 512], F32, tag="pva", bufs=2)
                        pvb = ppv.tile([D + 1, 512], F32, tag="pvb", bufs=2)
                        for kt in range(NKT):
                            ksl = slice(kt * 128, (kt + 1) * 128)
                            rg = kt % RING
                            stp = pst.tile([128, QW], F32, tag="st", bufs=2)
                            for qh in range(2):
                                nc.tensor.matmul(
                                    stp[:, qh * 512:(qh + 1) * 512],
                                    KTz[:, h, ksl],
                                    QT[:, pr, qbase + qh * 512:
                                       qbase + (qh + 1) * 512],
                                    start=True, stop=True)
                            if kt % 4 == 3:
                                # DVE Schraudolph fast-exp into fp16 bits
                                nc.vector.tensor_scalar(
                                    PT[:, rg, :].bitcast(I16),
                                    stp[:], SCH_A, SCH_B,
                                    op0=MULT, op1=ADD)
                            else:
                                nc.scalar.activation(
                                    PT[:, rg, :], stp[:], EXP, scale=0.125)
                            for qh, pvh in ((0, pva), (1, pvb)):
                                nc.tensor.matmul(
                                    pvh[:], Vn[:, kt, h, 0:65],
                                    PT[:, rg, qh * 512:(qh + 1) * 512],
                                    start=(kt == 0), stop=(kt == NKT - 1))
                        # previous head's normalization lands here, after
                        # this head's attention is queued
                        if pending is not None:
                            normalize(*pending)
                            # previous q-window's dense after its last
                            # head's normalization
                            if h == 1 and qw > 0:
                                dense_block(qw - 1)
                        pending = (qbase, h, pva, pvb)
                # dummy matmuls bridge the final normalization chain so the
                # HAM clock gate stays warm for the last dense block
                for _ in range(14):
                    dum = pst.tile([128, QW], F32, tag="st", bufs=2)
                    nc.tensor.matmul(dum[:, 0:512], wu[:, 0:128],
                                     wu[:, 128:640], start=True, stop=True)
                normalize(*pending)
                dense_block(S // QW - 1)

    nc.compile()
    return nc


_NC_CACHE = None


def get_nc():
    global _NC_CACHE
    if _NC_CACHE is None:
        _NC_CACHE = build_nc()
    return _NC_CACHE


def make_in_maps(hidden_states, w_qkv, b_qkv, w_dense):
    hs = np.asarray(hidden_states, dtype=np.float32)
    w_qkv = np.asarray(w_qkv, dtype=np.float32)
    b_qkv = np.asarray(b_qkv, dtype=np.float32)
    w_dense = np.asarray(w_dense, dtype=np.float32)
    # Reference layout: qkv.reshape(B, S, HEADS, 3*D) split on the last
    # axis, i.e. w_qkv columns are per-head [q_h | k_h | v_h] blocks of D.
    wq_cols = np.concatenate(
        [np.arange(h * 3 * D, h * 3 * D + D) for h in range(HEADS)])
    wk_cols = wq_cols + D
    wv_cols = wq_cols + 2 * D
    hst16 = [np.ascontiguousarray(hs[b].T).astype(np.float16)
             for b in range(B)]
    in_maps = []
    for c in range(NCORES):
        b = c // NHG
        hg = c % NHG
        sel = slice(hg * CW, (hg + 1) * CW)
        in_maps.append({
            "hst": hst16[b],
            "wq": np.ascontiguousarray(
                w_qkv[:, wq_cols[sel]]).astype(np.float16),
            "wk": np.ascontiguousarray(
                w_qkv[:, wk_cols[sel]]).astype(np.float16),
            "wv": np.ascontiguousarray(
                w_qkv[:, wv_cols[sel]]).astype(np.float16),
            "bq": np.ascontiguousarray(b_qkv[wq_cols[sel]].reshape(CW, 1)),
            "bk": np.ascontiguousarray(b_qkv[wk_cols[sel]].reshape(CW, 1)),
            "wd": np.ascontiguousarray(
                w_dense[sel, :]).astype(np.float16),
        })
    return in_maps


def run(hidden_states, w_qkv, b_qkv, w_dense, b_dense, trace=False):
    nc = get_nc()
    in_maps = make_in_maps(hidden_states, w_qkv, b_qkv, w_dense)
    res = run_bass_kernel_spmd(nc, in_maps, core_ids=list(range(NCORES)),
                               trace=trace)
    acc = np.zeros((B, S, HID), dtype=np.float32)
    for c in range(NCORES):
        acc[c // NHG] += res.results[c]["out"]
    # bias terms that commute to the end: v-bias through dense, dense bias
    b_qkv = np.asarray(b_qkv, dtype=np.float32)
    b_v = np.concatenate(
        [b_qkv[h * 3 * D + 2 * D:h * 3 * D + 3 * D] for h in range(HEADS)])
    acc = acc + (b_v @ np.asarray(w_dense, dtype=np.float32)
                 + np.asarray(b_dense, dtype=np.float32))
    return acc.astype(np.float32), res


def kernel(hidden_states, w_qkv, b_qkv, w_dense, b_dense):
    out, _ = run(hidden_states, w_qkv, b_qkv, w_dense, b_dense,
                 trace=bool(os.environ.get("BASS_TRACE")))
    return out


# revision 32
# speedup vs baseline: 1.0105x; 1.0105x over previous
"""Multi-head attention (B=2, S=2048, H=1024, 16 heads) on 8 NeuronCores.

Hybrid tensor-parallel sharding: core c handles batch c//4 and head-group
c%4 (4 heads).  Each core computes QKV for its heads over its batch, full
attention, and a partial output projection (its 256 rows of w_dense).  The
host sums the 4 partials per batch (the all-reduce) and adds the
output-side bias terms.

All matmul operands are fp16 (host pre-converts; ~8x the mantissa of bf16
at the same PE speed).  Accumulation is fp32 in PSUM.

Per-core layout:
  hsT  [128, 8, 512]    hidden states transposed (PE transpose), per
                        512-seq window.
  QT   [128, 2, 2048]   q transposed, packed: pair p rows = [h_even | h_odd].
  KTz  [128, 4, 2048]   k transposed per head, zero-padded so each head
                        contracts against the packed QT pair.
  Vn   [128, 16, 4, 66] v natural: partition = seq within 128-chunk;
                        col 64 is 1.0 so P@V also emits softmax rowsums.
  PT   [128, 4, 1024]   exp(scores) ring: partition = k within chunk.
                        3 of 4 tiles via ScalarE exp; every 4th via a
                        DVE Schraudolph fast-exp (int16 bit trick).
  ctxT [128, 2, 2048]   normalized context, packed like QT: the dense
                        matmul contracts full 128-row pair chunks.
  Normalization: P@V rowsum row -> reciprocal -> gpsimd partition
  broadcast -> one DVE multiply per psum tile (fuses the PSUM->SBUF copy).
A burst of dummy matmuls at t=0 warms the PE HAM clock gate while the
first DMAs land.
"""

import os
import sys
import types

sys.path.insert(0, "/opt/trn_rl_repo")

import numpy as np


def _install_ntff_shim():
    """The trimmed container image lacks ``antenv.axon_hooks``, which
    ``run_bass_kernel_spmd(trace=True)`` needs to capture NTFF profiles
    under axon.  Recreate it from the boot helper + the injected .so."""
    if "antenv.axon_hooks" in sys.modules:
        return
    try:
        from trn_agent_boot.trn_boot import _ntff_profile_via_ctypes
        so = "/opt/axon/libaxon_pjrt.so"
        if not os.path.exists(so):
            return
        hook = _ntff_profile_via_ctypes(so)
        mod = types.ModuleType("antenv.axon_hooks")
        mod.get_axon_ntff_profile_hook = lambda: hook
        mod.set_axon_ntff_profile_hook = lambda h: None
        sys.modules["antenv.axon_hooks"] = mod
    except Exception:
        pass


_install_ntff_shim()

import concourse.bass as bass
import concourse.mybir as mybir
import concourse.tile as tile
from concourse import bacc
from concourse.bass_utils import run_bass_kernel_spmd
from concourse.masks import make_identity

F32 = mybir.dt.float32
F16 = mybir.dt.float16
I16 = mybir.dt.int16
EXP = mybir.ActivationFunctionType.Exp
MULT = mybir.AluOpType.mult
ADD = mybir.AluOpType.add

B, S, HID = 2, 2048, 1024
HEADS, D = 16, 64
NCORES = 8
NBG = 2                          # batch groups
NHG = NCORES // NBG              # head groups = 4
HPC = HEADS // NHG               # heads per core = 4
NPAIR = HPC // 2                 # head pairs per core = 2
CW = HPC * D                     # per-core ctx width = 256
NHB = HID // 128                 # hidden 128-chunks = 8
WSEQ = 512                       # seq window for transpose+QKV
NWIN = S // WSEQ                 # 4
QW = 1024                        # q window in attention
NKT = S // 128                   # k chunks = 16
RING = 4

# fp16 Schraudolph fast-exp (DVE): i16 = round(x*SCH_A + SCH_B), bits
# reinterpreted as fp16 give exp(0.125*x) to ~3% max rel err.
SCH_A = 1024.0 * 1.4426950408889634 * 0.125   # 184.66496523378733
SCH_B = 15360.0 - 44.74                        # calibrated bias


def build_nc():
    nc = bacc.Bacc("TRN2", target_bir_lowering=False, debug=False,
                   num_devices=NCORES)

    hst = nc.dram_tensor("hst", [HID, S], F16, kind="ExternalInput")
    wq = nc.dram_tensor("wq", [HID, CW], F16, kind="ExternalInput")
    wk = nc.dram_tensor("wk", [HID, CW], F16, kind="ExternalInput")
    wv = nc.dram_tensor("wv", [HID, CW], F16, kind="ExternalInput")
    bq = nc.dram_tensor("bq", [CW, 1], F32, kind="ExternalInput")
    bk = nc.dram_tensor("bk", [CW, 1], F32, kind="ExternalInput")
    wd = nc.dram_tensor("wd", [CW, HID], F16, kind="ExternalInput")
    out = nc.dram_tensor("out", [S, HID], F32, kind="ExternalOutput")

    with tile.TileContext(nc) as tc:
        with (
            tc.tile_pool(name="persist", bufs=1) as pp,
            tc.tile_pool(name="pt", bufs=1) as ptp,
        ):
            # warmup scratch memset is the first gpsimd instruction so the
            # HAM-warming dummy matmuls can issue right after the preamble
            wu = pp.tile([128, 640], F16)
            nc.gpsimd.memset(wu[:], 0.0)

            ident = pp.tile([128, 128], F32)
            make_identity(nc, ident[:])
            identh_t = pp.tile([128, 128], F16)
            nc.vector.tensor_copy(identh_t[:], ident[:])
            identh = identh_t[:]

            wq_sb = pp.tile([128, NHB, CW], F16)
            wk_sb = pp.tile([128, NHB, CW], F16)
            wv_sb = pp.tile([128, NHB, CW], F16)
            for wsb, wdr in ((wq_sb, wq), (wk_sb, wk), (wv_sb, wv)):
                nc.gpsimd.dma_start(
                    wsb[:], wdr.ap().rearrange("(c p) m -> p c m", p=128))
            bq_sb = pp.tile([128, NPAIR], F32)
            bk_sb = pp.tile([128, NPAIR], F32)
            nc.gpsimd.dma_start(
                bq_sb[:], bq.ap().rearrange("(r p) m -> p (r m)", p=128))
            nc.gpsimd.dma_start(
                bk_sb[:], bk.ap().rearrange("(r p) m -> p (r m)", p=128))
            wd_sb = pp.tile([128, NPAIR, HID], F16)
            nc.gpsimd.dma_start(
                wd_sb[:], wd.ap().rearrange("(r p) m -> p r m", p=128))

            QT = pp.tile([128, NPAIR, S], F16)
            KTz = pp.tile([128, HPC, S], F16)
            Vn = pp.tile([128, NKT, HPC, 66], F16)
            ctxT = pp.tile([128, NPAIR, S], F16)
            PT = ptp.tile([128, RING, QW], F16)

            # zero-pad KTz: even head -> rows 64-127 zero, odd head ->
            # rows 0-63 zero; ones column for the P@V rowsum trick
            for h in range(HPC):
                if h % 2 == 0:
                    nc.vector.memset(KTz[D:128, h, :], 0.0)
                else:
                    nc.vector.memset(KTz[0:D, h, :], 0.0)
            nc.vector.memset(Vn[:, :, :, 64:65], 1.0)

            # ---------------- phase 1: load hs^T + QKV projections -------
            hsTf = pp.tile([128, NHB, S], F16)
            hst_re = hst.ap().rearrange("(c p) s -> p c s", p=128)
            with (
                tc.tile_pool(name="warm", bufs=1,
                             space=bass.MemorySpace.PSUM) as wup,
                tc.tile_pool(name="warms", bufs=1) as wsp,
                tc.tile_pool(name="vtw", bufs=2) as vwp,
                tc.tile_pool(name="ps_tr", bufs=2,
                             space=bass.MemorySpace.PSUM) as ptr,
                tc.tile_pool(name="ps_qkv", bufs=2,
                             space=bass.MemorySpace.PSUM) as pqk,
            ):
                # dummy matmuls at t=0: warm the HAM clock gate while the
                # first hs window DMAs in (PE is otherwise idle and cold)
                wups = wup.tile([128, 512], F32)
                for _ in range(52):
                    nc.tensor.matmul(wups[:], wu[:, 0:128], wu[:, 128:640],
                                     start=True, stop=True)

                for w in range(NWIN):
                    r0 = w * WSEQ
                    wsl = slice(r0, r0 + WSEQ)
                    nc.sync.dma_start(hsTf[:, :, wsl], hst_re[:, :, wsl])
                    for tgt in range(3):
                        for pr in range(NPAIR):
                            csl = slice(pr * 128, (pr + 1) * 128)
                            ps = pqk.tile([128, WSEQ], F32, tag="qkv")
                            wsb = (wq_sb, wk_sb, wv_sb)[tgt]
                            for hb in range(NHB):
                                nc.tensor.matmul(
                                    ps[:], wsb[:, hb, csl],
                                    hsTf[:, hb, wsl],
                                    start=(hb == 0), stop=(hb == NHB - 1))
                            if tgt == 0:
                                nc.vector.tensor_scalar_add(
                                    QT[:, pr, wsl], ps[:],
                                    bq_sb[:, pr:pr + 1])
                            elif tgt == 1:
                                nc.vector.tensor_scalar_add(
                                    KTz[0:D, 2 * pr, wsl], ps[0:D, :],
                                    bk_sb[0:D, pr:pr + 1])
                                nc.vector.tensor_scalar_add(
                                    KTz[D:128, 2 * pr + 1, wsl],
                                    ps[D:128, :],
                                    bk_sb[D:128, pr:pr + 1])
                            else:
                                vtw = vwp.tile([128, WSEQ], F16)
                                nc.vector.tensor_copy(vtw[:], ps[:])
                                vps = ptr.tile([128, WSEQ], F16, tag="vtr")
                                for sb2 in range(WSEQ // 128):
                                    nc.tensor.transpose(
                                        vps[:, sb2 * 128:(sb2 + 1) * 128],
                                        vtw[:, sb2 * 128:(sb2 + 1) * 128],
                                        identh)
                                ch0 = r0 // 128
                                nc.vector.tensor_copy(
                                    Vn[:, ch0:ch0 + 4,
                                       2 * pr:2 * pr + 2, 0:64],
                                    vps[:].rearrange(
                                        "p (c h d) -> p c h d", c=4, h=2))

            # ---------------- phase 2: attention + output projection -----
            with (
                tc.tile_pool(name="ps_st", bufs=1,
                             space=bass.MemorySpace.PSUM) as pst,
                tc.tile_pool(name="ps_pv", bufs=1,
                             space=bass.MemorySpace.PSUM) as ppv,
                tc.tile_pool(name="dens", bufs=2) as dnp,
                tc.tile_pool(name="outst", bufs=4) as osp,
            ):
                def dense_block(dqw):
                    # packed pair contraction, out natural [q, oc]
                    dqbase = dqw * QW
                    for qc in range(QW // 128):
                        ssl = slice(dqbase + qc * 128,
                                    dqbase + (qc + 1) * 128)
                        for nt in range(2):
                            nsl = slice(nt * 512, (nt + 1) * 512)
                            dpw = pst.tile([128, QW], F32, tag="st", bufs=2)
                            dps = dpw[:, 0:512]
                            for pr in range(NPAIR):
                                nc.tensor.matmul(
                                    dps, ctxT[:, pr, ssl],
                                    wd_sb[:, pr, nsl],
                                    start=(pr == 0), stop=(pr == NPAIR - 1))
                            ob = osp.tile([128, 512], F32)
                            nc.vector.tensor_copy(ob[:], dps)
                            nc.sync.dma_start(out[ssl, nsl], ob[:])

                def norm_recip(pva, pvb):
                    # rowsum rows -> approx reciprocal -> gpsimd partition
                    # broadcast (emitted right after the head's attention;
                    # the broadcast completes during the NEXT head)
                    den = dnp.tile([1, QW], F32, tag="den")
                    nc.vector.tensor_copy(den[0:1, 0:512], pva[D:D + 1, :])
                    nc.vector.tensor_copy(den[0:1, 512:QW], pvb[D:D + 1, :])
                    rden = dnp.tile([1, QW], F32, tag="rden")
                    nc.vector.reciprocal_approx_fast(rden[:], den[:])
                    rbc = dnp.tile([D, QW], F32, tag="rbc")
                    nc.gpsimd.partition_broadcast(
                        rbc[:], rden[0:1, :], channels=D)
                    return rbc

                def norm_mults(qbase, h, pva, pvb, rbc):
                    # the normalization multiplies, one head behind, so the
                    # DVE FIFO never waits on the gpsimd broadcast
                    pr = h // 2
                    hr = slice((h % 2) * D, (h % 2) * D + D)
                    for qh, pvh in ((0, pva), (1, pvb)):
                        nc.vector.tensor_tensor(
                            ctxT[hr, pr, qbase + qh * 512:
                                 qbase + (qh + 1) * 512],
                            pvh[0:D, :],
                            rbc[:, qh * 512:(qh + 1) * 512],
                            op=MULT)

                pending = None
                for qw in range(S // QW):
                    qbase = qw * QW
                    for h in range(HPC):
                        pr = h // 2
                        hr = slice((h % 2) * D, (h % 2) * D + D)
                        pva = ppv.tile([D + 1, 512], F32, tag="pva", bufs=2)
                        pvb = ppv.tile([D + 1, 512], F32, tag="pvb", bufs=2)
                        for kt in range(NKT):
                            ksl = slice(kt * 128, (kt + 1) * 128)
                            rg = kt % RING
                            stp = pst.tile([128, QW], F32, tag="st", bufs=2)
                            for qh in range(2):
                                nc.tensor.matmul(
                                    stp[:, qh * 512:(qh + 1) * 512],
                                    KTz[:, h, ksl],
                                    QT[:, pr, qbase + qh * 512:
                                       qbase + (qh + 1) * 512],
                                    start=True, stop=True)
                            if kt % 4 == 3:
                                # DVE Schraudolph fast-exp into fp16 bits
                                nc.vector.tensor_scalar(
                                    PT[:, rg, :].bitcast(I16),
                                    stp[:], SCH_A, SCH_B,
                                    op0=MULT, op1=ADD)
                            else:
                                nc.scalar.activation(
                                    PT[:, rg, :], stp[:], EXP, scale=0.125)
                            for qh, pvh in ((0, pva), (1, pvb)):
                                nc.tensor.matmul(
                                    pvh[:], Vn[:, kt, h, 0:65],
                                    PT[:, rg, qh * 512:(qh + 1) * 512],
                                    start=(kt == 0), stop=(kt == NKT - 1))
                        # previous head's normalization multiplies land
                        # here; this head's recip+broadcast start now
                        if pending is not None:
                            norm_mults(*pending)
                            # previous q-window's dense after its last
                            # head's normalization multiplies
                            if h == 1 and qw > 0:
                                dense_block(qw - 1)
                        rbc = norm_recip(pva, pvb)
                        pending = (qbase, h, pva, pvb, rbc)
                # dummy matmuls bridge the final normalization chain so the
                # HAM clock gate stays warm for the last dense block
                for _ in range(14):
                    dum = pst.tile([128, QW], F32, tag="st", bufs=2)
                    nc.tensor.matmul(dum[:, 0:512], wu[:, 0:128],
                                     wu[:, 128:640], start=True, stop=True)
                norm_mults(*pending)
                dense_block(S // QW - 1)

    nc.compile()
    return nc


_NC_CACHE = None


def get_nc():
    global _NC_CACHE
    if _NC_CACHE is None:
        _NC_CACHE = build_nc()
    return _NC_CACHE


def make_in_maps(hidden_states, w_qkv, b_qkv, w_dense):
    hs = np.asarray(hidden_states, dtype=np.float32)
    w_qkv = np.asarray(w_qkv, dtype=np.float32)
    b_qkv = np.asarray(b_qkv, dtype=np.float32)
    w_dense = np.asarray(w_dense, dtype=np.float32)
    # Reference layout: qkv.reshape(B, S, HEADS, 3*D) split on the last
    # axis, i.e. w_qkv columns are per-head [q_h | k_h | v_h] blocks of D.
    wq_cols = np.concatenate(
        [np.arange(h * 3 * D, h * 3 * D + D) for h in range(HEADS)])
    wk_cols = wq_cols + D
    wv_cols = wq_cols + 2 * D
    hst16 = [np.ascontiguousarray(hs[b].T).astype(np.float16)
             for b in range(B)]
    in_maps = []
    for c in range(NCORES):
        b = c // NHG
        hg = c % NHG
        sel = slice(hg * CW, (hg + 1) * CW)
        in_maps.append({
            "hst": hst16[b],
            "wq": np.ascontiguousarray(
                w_qkv[:, wq_cols[sel]]).astype(np.float16),
            "wk": np.ascontiguousarray(
                w_qkv[:, wk_cols[sel]]).astype(np.float16),
            "wv": np.ascontiguousarray(
                w_qkv[:, wv_cols[sel]]).astype(np.float16),
            "bq": np.ascontiguousarray(b_qkv[wq_cols[sel]].reshape(CW, 1)),
            "bk": np.ascontiguousarray(b_qkv[wk_cols[sel]].reshape(CW, 1)),
            "wd": np.ascontiguousarray(
                w_dense[sel, :]).astype(np.float16),
        })
    return in_maps


def run(hidden_states, w_qkv, b_qkv, w_dense, b_dense, trace=False):
    nc = get_nc()
    in_maps = make_in_maps(hidden_states, w_qkv, b_qkv, w_dense)
    res = run_bass_kernel_spmd(nc, in_maps, core_ids=list(range(NCORES)),
                               trace=trace)
    acc = np.zeros((B, S, HID), dtype=np.float32)
    for c in range(NCORES):
        acc[c // NHG] += res.results[c]["out"]
    # bias terms that commute to the end: v-bias through dense, dense bias
    b_qkv = np.asarray(b_qkv, dtype=np.float32)
    b_v = np.concatenate(
        [b_qkv[h * 3 * D + 2 * D:h * 3 * D + 3 * D] for h in range(HEADS)])
    acc = acc + (b_v @ np.asarray(w_dense, dtype=np.float32)
                 + np.asarray(b_dense, dtype=np.float32))
    return acc.astype(np.float32), res


def kernel(hidden_states, w_qkv, b_qkv, w_dense, b_dense):
    out, _ = run(hidden_states, w_qkv, b_qkv, w_dense, b_dense,
                 trace=bool(os.environ.get("BASS_TRACE")))
    return out


# revision 35
# speedup vs baseline: 1.0152x; 1.0046x over previous
"""Multi-head attention (B=2, S=2048, H=1024, 16 heads) on 8 NeuronCores.

Hybrid tensor-parallel sharding: core c handles batch c//4 and head-group
c%4 (4 heads).  Each core computes QKV for its heads over its batch, full
attention, and a partial output projection (its 256 rows of w_dense).  The
host sums the 4 partials per batch (the all-reduce) and adds the
output-side bias terms.

All matmul operands are fp16 (host pre-converts; ~8x the mantissa of bf16
at the same PE speed).  Accumulation is fp32 in PSUM.

Per-core layout:
  hsT  [128, 8, 512]    hidden states transposed (PE transpose), per
                        512-seq window.
  QT   [128, 2, 2048]   q transposed, packed: pair p rows = [h_even | h_odd].
  KTz  [128, 4, 2048]   k transposed per head, zero-padded so each head
                        contracts against the packed QT pair.
  Vn   [128, 16, 4, 66] v natural: partition = seq within 128-chunk;
                        col 64 is 1.0 so P@V also emits softmax rowsums.
  PT   [128, 4, 1024]   exp(scores) ring: partition = k within chunk.
                        3 of 4 tiles via ScalarE exp; every 4th via a
                        DVE Schraudolph fast-exp (int16 bit trick).
  ctxT [128, 2, 2048]   normalized context, packed like QT: the dense
                        matmul contracts full 128-row pair chunks.
  Normalization: P@V rowsum row -> reciprocal -> gpsimd partition
  broadcast -> one DVE multiply per psum tile (fuses the PSUM->SBUF copy).
A burst of dummy matmuls at t=0 warms the PE HAM clock gate while the
first DMAs land.
"""

import os
import sys
import types

sys.path.insert(0, "/opt/trn_rl_repo")

import numpy as np


def _install_ntff_shim():
    """The trimmed container image lacks ``antenv.axon_hooks``, which
    ``run_bass_kernel_spmd(trace=True)`` needs to capture NTFF profiles
    under axon.  Recreate it from the boot helper + the injected .so."""
    if "antenv.axon_hooks" in sys.modules:
        return
    try:
        from trn_agent_boot.trn_boot import _ntff_profile_via_ctypes
        so = "/opt/axon/libaxon_pjrt.so"
        if not os.path.exists(so):
            return
        hook = _ntff_profile_via_ctypes(so)
        mod = types.ModuleType("antenv.axon_hooks")
        mod.get_axon_ntff_profile_hook = lambda: hook
        mod.set_axon_ntff_profile_hook = lambda h: None
        sys.modules["antenv.axon_hooks"] = mod
    except Exception:
        pass


_install_ntff_shim()

import concourse.bass as bass
import concourse.mybir as mybir
import concourse.tile as tile
from concourse import bacc
from concourse.bass_utils import run_bass_kernel_spmd
from concourse.masks import make_identity

F32 = mybir.dt.float32
F16 = mybir.dt.float16
I16 = mybir.dt.int16
EXP = mybir.ActivationFunctionType.Exp
MULT = mybir.AluOpType.mult
ADD = mybir.AluOpType.add

B, S, HID = 2, 2048, 1024
HEADS, D = 16, 64
NCORES = 8
NBG = 2                          # batch groups
NHG = NCORES // NBG              # head groups = 4
HPC = HEADS // NHG               # heads per core = 4
NPAIR = HPC // 2                 # head pairs per core = 2
CW = HPC * D                     # per-core ctx width = 256
NHB = HID // 128                 # hidden 128-chunks = 8
WSEQ = 512                       # seq window for transpose+QKV
NWIN = S // WSEQ                 # 4
QW = 1024                        # q window in attention
NKT = S // 128                   # k chunks = 16
RING = 4

# fp16 Schraudolph fast-exp (DVE): i16 = round(x*SCH_A + SCH_B), bits
# reinterpreted as fp16 give exp(0.125*x) to ~3% max rel err.
SCH_A = 1024.0 * 1.4426950408889634 * 0.125   # 184.66496523378733
SCH_B = 15360.0 - 44.74                        # calibrated bias


def build_nc():
    nc = bacc.Bacc("TRN2", target_bir_lowering=False, debug=False,
                   num_devices=NCORES)

    hst = nc.dram_tensor("hst", [HID, S], F16, kind="ExternalInput")
    wq = nc.dram_tensor("wq", [HID, CW], F16, kind="ExternalInput")
    wk = nc.dram_tensor("wk", [HID, CW], F16, kind="ExternalInput")
    wv = nc.dram_tensor("wv", [HID, CW], F16, kind="ExternalInput")
    bq = nc.dram_tensor("bq", [CW, 1], F32, kind="ExternalInput")
    bk = nc.dram_tensor("bk", [CW, 1], F32, kind="ExternalInput")
    wd = nc.dram_tensor("wd", [CW, HID], F16, kind="ExternalInput")
    out = nc.dram_tensor("out", [S, HID], F32, kind="ExternalOutput")

    with tile.TileContext(nc) as tc:
        with (
            tc.tile_pool(name="persist", bufs=1) as pp,
            tc.tile_pool(name="pt", bufs=1) as ptp,
        ):
            # warmup scratch memset is the first gpsimd instruction so the
            # HAM-warming dummy matmuls can issue right after the preamble
            wu = pp.tile([128, 640], F16)
            nc.gpsimd.memset(wu[:], 0.0)

            ident = pp.tile([128, 128], F32)
            make_identity(nc, ident[:])
            identh_t = pp.tile([128, 128], F16)
            nc.vector.tensor_copy(identh_t[:], ident[:])
            identh = identh_t[:]

            wq_sb = pp.tile([128, NHB, CW], F16)
            wk_sb = pp.tile([128, NHB, CW], F16)
            wv_sb = pp.tile([128, NHB, CW], F16)
            for wsb, wdr in ((wq_sb, wq), (wk_sb, wk), (wv_sb, wv)):
                nc.gpsimd.dma_start(
                    wsb[:], wdr.ap().rearrange("(c p) m -> p c m", p=128))
            bq_sb = pp.tile([128, NPAIR], F32)
            bk_sb = pp.tile([128, NPAIR], F32)
            nc.gpsimd.dma_start(
                bq_sb[:], bq.ap().rearrange("(r p) m -> p (r m)", p=128))
            nc.gpsimd.dma_start(
                bk_sb[:], bk.ap().rearrange("(r p) m -> p (r m)", p=128))
            wd_sb = pp.tile([128, NPAIR, HID], F16)
            nc.gpsimd.dma_start(
                wd_sb[:], wd.ap().rearrange("(r p) m -> p r m", p=128))

            QT = pp.tile([128, NPAIR, S], F16)
            KTz = pp.tile([128, HPC, S], F16)
            Vn = pp.tile([128, NKT, HPC, 66], F16)
            ctxT = pp.tile([128, NPAIR, S], F16)
            PT = ptp.tile([128, RING, QW], F16)

            # zero-pad KTz: even head -> rows 64-127 zero, odd head ->
            # rows 0-63 zero; ones column for the P@V rowsum trick
            for h in range(HPC):
                if h % 2 == 0:
                    nc.vector.memset(KTz[D:128, h, :], 0.0)
                else:
                    nc.vector.memset(KTz[0:D, h, :], 0.0)
            nc.vector.memset(Vn[:, :, :, 64:65], 1.0)

            # ---------------- phase 1: load hs^T + QKV projections -------
            hsTf = pp.tile([128, NHB, S], F16)
            hst_re = hst.ap().rearrange("(c p) s -> p c s", p=128)
            with (
                tc.tile_pool(name="warm", bufs=1,
                             space=bass.MemorySpace.PSUM) as wup,
                tc.tile_pool(name="warms", bufs=1) as wsp,
                tc.tile_pool(name="vtw", bufs=2) as vwp,
                tc.tile_pool(name="ps_tr", bufs=2,
                             space=bass.MemorySpace.PSUM) as ptr,
                tc.tile_pool(name="ps_qkv", bufs=2,
                             space=bass.MemorySpace.PSUM) as pqk,
            ):
                # dummy matmuls at t=0: warm the HAM clock gate while the
                # first hs window DMAs in (PE is otherwise idle and cold)
                wups = wup.tile([128, 512], F32)
                for _ in range(52):
                    nc.tensor.matmul(wups[:], wu[:, 0:128], wu[:, 128:640],
                                     start=True, stop=True)

                for w in range(NWIN):
                    r0 = w * WSEQ
                    wsl = slice(r0, r0 + WSEQ)
                    nc.sync.dma_start(hsTf[:, :, wsl], hst_re[:, :, wsl])
                    for tgt in range(3):
                        for pr in range(NPAIR):
                            csl = slice(pr * 128, (pr + 1) * 128)
                            ps = pqk.tile([128, WSEQ], F32, tag="qkv")
                            wsb = (wq_sb, wk_sb, wv_sb)[tgt]
                            for hb in range(NHB):
                                nc.tensor.matmul(
                                    ps[:], wsb[:, hb, csl],
                                    hsTf[:, hb, wsl],
                                    start=(hb == 0), stop=(hb == NHB - 1))
                            if tgt == 0:
                                nc.vector.tensor_scalar_add(
                                    QT[:, pr, wsl], ps[:],
                                    bq_sb[:, pr:pr + 1])
                            elif tgt == 1:
                                nc.vector.tensor_scalar_add(
                                    KTz[0:D, 2 * pr, wsl], ps[0:D, :],
                                    bk_sb[0:D, pr:pr + 1])
                                nc.vector.tensor_scalar_add(
                                    KTz[D:128, 2 * pr + 1, wsl],
                                    ps[D:128, :],
                                    bk_sb[D:128, pr:pr + 1])
                            else:
                                vtw = vwp.tile([128, WSEQ], F16)
                                nc.vector.tensor_copy(vtw[:], ps[:])
                                vps = ptr.tile([128, WSEQ], F16, tag="vtr")
                                for sb2 in range(WSEQ // 128):
                                    nc.tensor.transpose(
                                        vps[:, sb2 * 128:(sb2 + 1) * 128],
                                        vtw[:, sb2 * 128:(sb2 + 1) * 128],
                                        identh)
                                ch0 = r0 // 128
                                nc.vector.tensor_copy(
                                    Vn[:, ch0:ch0 + 4,
                                       2 * pr:2 * pr + 2, 0:64],
                                    vps[:].rearrange(
                                        "p (c h d) -> p c h d", c=4, h=2))

            # ---------------- phase 2: attention + output projection -----
            with (
                tc.tile_pool(name="ps_st", bufs=1,
                             space=bass.MemorySpace.PSUM) as pst,
                tc.tile_pool(name="ps_pv", bufs=1,
                             space=bass.MemorySpace.PSUM) as ppv,
                tc.tile_pool(name="dens", bufs=2) as dnp,
                tc.tile_pool(name="outst", bufs=4) as osp,
            ):
                def dense_block(dqw):
                    # packed pair contraction, out natural [q, oc]
                    dqbase = dqw * QW
                    for qc in range(QW // 128):
                        ssl = slice(dqbase + qc * 128,
                                    dqbase + (qc + 1) * 128)
                        for nt in range(2):
                            nsl = slice(nt * 512, (nt + 1) * 512)
                            dpw = pst.tile([128, QW], F32, tag="st", bufs=2)
                            dps = dpw[:, 0:512]
                            for pr in range(NPAIR):
                                nc.tensor.matmul(
                                    dps, ctxT[:, pr, ssl],
                                    wd_sb[:, pr, nsl],
                                    start=(pr == 0), stop=(pr == NPAIR - 1))
                            ob = osp.tile([128, 512], F32)
                            if nt == 0:
                                nc.scalar.copy(ob[:], dps)
                            else:
                                nc.vector.tensor_copy(ob[:], dps)
                            nc.sync.dma_start(out[ssl, nsl], ob[:])

                def norm_recip(pva, pvb):
                    # rowsum rows -> approx reciprocal -> gpsimd partition
                    # broadcast (emitted right after the head's attention;
                    # the broadcast completes during the NEXT head)
                    den = dnp.tile([1, QW], F32, tag="den")
                    nc.vector.tensor_copy(den[0:1, 0:512], pva[D:D + 1, :])
                    nc.vector.tensor_copy(den[0:1, 512:QW], pvb[D:D + 1, :])
                    rden = dnp.tile([1, QW], F32, tag="rden")
                    nc.vector.reciprocal_approx_fast(rden[:], den[:])
                    rbc = dnp.tile([D, QW], F32, tag="rbc")
                    nc.gpsimd.partition_broadcast(
                        rbc[:], rden[0:1, :], channels=D)
                    return rbc

                def norm_mults(qbase, h, pva, pvb, rbc):
                    # the normalization multiplies, one head behind, so the
                    # DVE FIFO never waits on the gpsimd broadcast
                    pr = h // 2
                    hr = slice((h % 2) * D, (h % 2) * D + D)
                    for qh, pvh in ((0, pva), (1, pvb)):
                        nc.vector.tensor_tensor(
                            ctxT[hr, pr, qbase + qh * 512:
                                 qbase + (qh + 1) * 512],
                            pvh[0:D, :],
                            rbc[:, qh * 512:(qh + 1) * 512],
                            op=MULT)

                pending = None
                for qw in range(S // QW):
                    qbase = qw * QW
                    for h in range(HPC):
                        pr = h // 2
                        hr = slice((h % 2) * D, (h % 2) * D + D)
                        pva = ppv.tile([D + 1, 512], F32, tag="pva", bufs=2)
                        pvb = ppv.tile([D + 1, 512], F32, tag="pvb", bufs=2)
                        for kt in range(NKT):
                            ksl = slice(kt * 128, (kt + 1) * 128)
                            rg = kt % RING
                            stp = pst.tile([128, QW], F32, tag="st", bufs=2)
                            for qh in range(2):
                                nc.tensor.matmul(
                                    stp[:, qh * 512:(qh + 1) * 512],
                                    KTz[:, h, ksl],
                                    QT[:, pr, qbase + qh * 512:
                                       qbase + (qh + 1) * 512],
                                    start=True, stop=True)
                            if kt % 4 == 3:
                                # DVE Schraudolph fast-exp into fp16 bits
                                nc.vector.tensor_scalar(
                                    PT[:, rg, :].bitcast(I16),
                                    stp[:], SCH_A, SCH_B,
                                    op0=MULT, op1=ADD)
                            else:
                                nc.scalar.activation(
                                    PT[:, rg, :], stp[:], EXP, scale=0.125)
                            for qh, pvh in ((0, pva), (1, pvb)):
                                nc.tensor.matmul(
                                    pvh[:], Vn[:, kt, h, 0:65],
                                    PT[:, rg, qh * 512:(qh + 1) * 512],
                                    start=(kt == 0), stop=(kt == NKT - 1))
                        # previous head's normalization multiplies land
                        # here; this head's recip+broadcast start now
                        if pending is not None:
                            norm_mults(*pending)
                            # previous q-window's dense after its last
                            # head's normalization multiplies
                            if h == 1 and qw > 0:
                                dense_block(qw - 1)
                        rbc = norm_recip(pva, pvb)
                        pending = (qbase, h, pva, pvb, rbc)
                # dummy matmuls bridge the final normalization chain so the
                # HAM clock gate stays warm for the last dense block; the
                # rbc-reading ones only become ready mid-chain, after the
                # gpsimd broadcast, spanning the PE-idle window
                for _ in range(8):
                    dum = pst.tile([128, QW], F32, tag="st", bufs=2)
                    nc.tensor.matmul(dum[:, 0:512], wu[:, 0:128],
                                     wu[:, 128:640], start=True, stop=True)
                rbcf = pending[4]
                for j in range(6):
                    dum = pst.tile([128, QW], F32, tag="st", bufs=2)
                    nc.tensor.matmul(dum[:, 0:512], wu[0:D, 0:128],
                                     rbcf[:, 0:256].bitcast(F16),
                                     start=True, stop=True)
                norm_mults(*pending)
                dense_block(S // QW - 1)

    nc.compile()
    return nc


_NC_CACHE = None


def get_nc():
    global _NC_CACHE
    if _NC_CACHE is None:
        _NC_CACHE = build_nc()
    return _NC_CACHE


def make_in_maps(hidden_states, w_qkv, b_qkv, w_dense):
    hs = np.asarray(hidden_states, dtype=np.float32)
    w_qkv = np.asarray(w_qkv, dtype=np.float32)
    b_qkv = np.asarray(b_qkv, dtype=np.float32)
    w_dense = np.asarray(w_dense, dtype=np.float32)
    # Reference layout: qkv.reshape(B, S, HEADS, 3*D) split on the last
    # axis, i.e. w_qkv columns are per-head [q_h | k_h | v_h] blocks of D.
    wq_cols = np.concatenate(
        [np.arange(h * 3 * D, h * 3 * D + D) for h in range(HEADS)])
    wk_cols = wq_cols + D
    wv_cols = wq_cols + 2 * D
    hst16 = [np.ascontiguousarray(hs[b].T).astype(np.float16)
             for b in range(B)]
    in_maps = []
    for c in range(NCORES):
        b = c // NHG
        hg = c % NHG
        sel = slice(hg * CW, (hg + 1) * CW)
        in_maps.append({
            "hst": hst16[b],
            "wq": np.ascontiguousarray(
                w_qkv[:, wq_cols[sel]]).astype(np.float16),
            "wk": np.ascontiguousarray(
                w_qkv[:, wk_cols[sel]]).astype(np.float16),
            "wv": np.ascontiguousarray(
                w_qkv[:, wv_cols[sel]]).astype(np.float16),
            "bq": np.ascontiguousarray(b_qkv[wq_cols[sel]].reshape(CW, 1)),
            "bk": np.ascontiguousarray(b_qkv[wk_cols[sel]].reshape(CW, 1)),
            "wd": np.ascontiguousarray(
                w_dense[sel, :]).astype(np.float16),
        })
    return in_maps


def run(hidden_states, w_qkv, b_qkv, w_dense, b_dense, trace=False):
    nc = get_nc()
    in_maps = make_in_maps(hidden_states, w_qkv, b_qkv, w_dense)
    res = run_bass_kernel_spmd(nc, in_maps, core_ids=list(range(NCORES)),
                               trace=trace)
    acc = np.zeros((B, S, HID), dtype=np.float32)
    for c in range(NCORES):
        acc[c // NHG] += res.results[c]["out"]
    # bias terms that commute to the end: v-bias through dense, dense bias
    b_qkv = np.asarray(b_qkv, dtype=np.float32)
    b_v = np.concatenate(
        [b_qkv[h * 3 * D + 2 * D:h * 3 * D + 3 * D] for h in range(HEADS)])
    acc = acc + (b_v @ np.asarray(w_dense, dtype=np.float32)
                 + np.asarray(b_dense, dtype=np.float32))
    return acc.astype(np.float32), res


def kernel(hidden_states, w_qkv, b_qkv, w_dense, b_dense):
    out, _ = run(hidden_states, w_qkv, b_qkv, w_dense, b_dense,
                 trace=bool(os.environ.get("BASS_TRACE")))
    return out
